# revision 4
# baseline (speedup 1.0000x reference)
"""Multi-head attention kernel for Trainium2, 8 NeuronCores.

Problem: B=2, S=2048, D=1024, H=16, Dk=64, fp32.
  qkv = x @ W_qkv + b_qkv ; per-head scaled-dot-product attention with
  key mask; out = attn_out @ W_out + b_out.

Sharding: DP over batch (2) x TP over head groups (4 groups of 4 heads).
Core c -> (b = c // 4, g = c % 4). Each core computes the partial output
  y_partial[b] = attn_out[:, heads(g)] @ W_out[rows(g)]
and the host sums the 4 partials per batch and adds b_out.

v2 design (bf16 datapath, ACT reserved for exp only):
  - All SBUF operands bf16 (PSUM accumulation fp32): Q^T/K^T/V/e^T/out^T
    and all weights. End-to-end rel err ~5e-3 vs the 2e-2 gate.
  - K bias dropped entirely: scores terms independent of the key cancel
    in softmax. Q bias added on DVE (per-partition tensor_scalar_add)
    during the PSUM->SBUF copy of Q^T. ACT runs nothing but exp.
  - Zero-padding of the per-head Q^T/K^T stripes (dk=64 padded to 128
    contraction rows) is hoisted OUTSIDE the repeat loop: pad rows are
    never written inside the loop, so one memset persists.
  - Phase B: for each q-chunk (512) x head-pair: 16 key-tiles, each
    2 score MMs (N=512) -> one exp [128,1024] (bias = per-key mask,
    scale 1/8) -> 2 accumulate MMs onto [65,512] PSUM (ones column of
    V_aug produces the softmax denominator). PSUM: 2x2-bank score bufs
    + 2x1-bank accumulators + 2 aux banks for phases A/C.
  - Phase C (out-proj of a finished q-chunk) is emitted between phase-B
    blocks so the PE fills the gap while ACT drains exps.
  - Normalization reads PSUM directly: reciprocal of the denominator
    row, gpsimd partition_broadcast, DVE multiply -> out^T bf16.
"""

import numpy as np
from contextlib import ExitStack

import concourse.tile as tile
from concourse import bacc, mybir
from concourse.bass_utils import run_bass_kernel_spmd

F32 = mybir.dt.float32
BF16 = mybir.dt.bfloat16
AF = mybir.ActivationFunctionType

S = 2048
D = 1024
H_LOC = 4           # heads per core
DK = 64
DH = H_LOC * DK     # 256: d' per core
KT = D // 128       # 8 k-tiles for the D contraction
ST = S // 128       # 16 s-tiles
SC = 4              # s super-chunks of 512
QC = 4              # q chunks of 512 in phase B
INV_SCALE = 1.0 / 8.0

TRACE = False
TRACE_ALL_CORES = False
LAST_EXEC_NS = None
LAST_RESULTS = None
LAST_IN_MAPS = None

_CACHED_NC = None


def _build(repeat=1):
    nc = bacc.Bacc("TRN2", target_bir_lowering=False, debug=False,
                   enable_asserts=True, num_devices=8)

    x = nc.dram_tensor("x", [S, D], F32, kind="ExternalInput").ap()
    w_q = nc.dram_tensor("w_q", [D, DH], F32, kind="ExternalInput").ap()
    w_k = nc.dram_tensor("w_k", [D, DH], F32, kind="ExternalInput").ap()
    w_v = nc.dram_tensor("w_v", [D, DH], F32, kind="ExternalInput").ap()
    b_q = nc.dram_tensor("b_q", [DH], F32, kind="ExternalInput").ap()
    w_out = nc.dram_tensor("w_out", [DH, D], F32, kind="ExternalInput").ap()
    mask_bias = nc.dram_tensor("mask_bias", [S], F32, kind="ExternalInput").ap()
    bv_bc_in = nc.dram_tensor("bv_bc", [128, DH], F32, kind="ExternalInput").ap()
    ident_in = nc.dram_tensor("ident", [128, 128], F32, kind="ExternalInput").ap()

    y = nc.dram_tensor("y", [S, D], BF16, kind="ExternalOutput").ap()

    with tile.TileContext(nc) as tc, ExitStack() as ctx:
        # ---------- persistent SBUF (handles created outside the loop) ----
        persist = ctx.enter_context(tc.tile_pool(name="persist", bufs=1))

        # Q^T / K^T per head, zero-padded to 128 contraction rows: head
        # 2m+a keeps its natural partitions (a=0 -> rows 0:64 real,
        # 64:128 zero; a=1 -> rows 64:128 real, 0:64 zero).
        qt = persist.tile([128, H_LOC, S], BF16, tag="qt")
        kt_sb = persist.tile([128, H_LOC, S], BF16, tag="kt")
        v_aug = persist.tile([128, ST, H_LOC, DK + 1], BF16, tag="vaug")
        out_ht = persist.tile([128, 2, S], BF16, tag="outht")  # attn out^T
        w_out_sb = persist.tile([128, 2, D], BF16, tag="wout")
        wq_sb = persist.tile([128, KT, DH], BF16, tag="wq")
        wk_sb = persist.tile([128, KT, DH], BF16, tag="wk")
        wv_sb = persist.tile([128, KT, DH], BF16, tag="wv")
        bq_sb = persist.tile([128, 2], F32, tag="bq")
        bv_bc = persist.tile([128, DH], F32, tag="bvbc")
        mask_sb = persist.tile([128, ST], F32, tag="mask")
        ident = persist.tile([128, 128], F32, tag="ident")

        # ---- one-time setup, hoisted out of the repeat loop ----
        # pad halves of the Q^T/K^T head stripes stay zero forever (the
        # in-loop copies write only the 64 real rows)
        for h in range(H_LOC):
            lo, hi = (64, 128) if h % 2 == 0 else (0, 64)
            nc.vector.memset(qt[lo:hi, h, :], 0.0)
            nc.vector.memset(kt_sb[lo:hi, h, :], 0.0)
        # ones column of V_aug (softmax denominator trick)
        nc.vector.memset(v_aug[:, :, :, DK:DK + 1], 1.0)

        if repeat > 1:
            ctx.enter_context(tc.For_i(0, repeat, 1))

        # ---------- per-iteration input DMA ----------
        nc.sync.dma_start(ident[:], ident_in)
        nc.sync.dma_start(bq_sb[:], b_q.rearrange("(m p) -> p m", p=128))
        nc.sync.dma_start(mask_sb[:], mask_bias.rearrange("(t p) -> p t", p=128))
        nc.sync.dma_start(bv_bc[:], bv_bc_in)

        with ExitStack() as body:
            wstage = body.enter_context(tc.tile_pool(name="wstage", bufs=2))
            xload = body.enter_context(tc.tile_pool(name="xload", bufs=3))
            xtp = body.enter_context(tc.tile_pool(name="xtp", bufs=2))
            epool = body.enter_context(tc.tile_pool(name="expt", bufs=4))
            small = body.enter_context(tc.tile_pool(name="small", bufs=4))
            ypool = body.enter_context(tc.tile_pool(name="ypool", bufs=3))
            aux = body.enter_context(tc.tile_pool(name="aux", bufs=2,
                                                  space="PSUM"))
            sps = body.enter_context(tc.tile_pool(name="sps", bufs=2,
                                                  space="PSUM"))
            ops = body.enter_context(tc.tile_pool(name="ops", bufs=2,
                                                  space="PSUM"))

            # weight loads: fast f32 HWDGE DMA into staging, convert bf16
            for wi, (wt, wd) in enumerate(((wq_sb, w_q), (wk_sb, w_k),
                                           (wv_sb, w_v))):
                w_stage = wstage.tile([128, KT, DH], F32, tag="wstage",
                                      name=f"wstage{wi}")
                nc.sync.dma_start(w_stage[:],
                                  wd.rearrange("(t p) d -> p t d", p=128))
                eng = nc.vector if wi % 2 == 0 else nc.gpsimd
                eng.tensor_copy(wt[:], w_stage[:])
            wo_stage = wstage.tile([128, 2, D], F32, tag="wostage")
            nc.sync.dma_start(wo_stage[:],
                              w_out.rearrange("(t p) d -> p t d", p=128))
            nc.gpsimd.tensor_copy(w_out_sb[:], wo_stage[:])

            # ---------- phase A: x^T, QKV projections ----------
            for sc in range(SC):
                # transpose 512 rows of x into xt_buf [128, kt, 512] bf16
                xt_buf = xtp.tile([128, KT, 512], BF16, tag="xt")
                for st4 in range(4):
                    sti = sc * 4 + st4
                    x_sb = xload.tile([128, D], F32, tag="x")
                    nc.sync.dma_start(x_sb[:], x[sti * 128:(sti + 1) * 128, :])
                    for kg in range(2):
                        p_t = aux.tile([128, 4, 128], F32, tag="aux", name="p_t")
                        for kj in range(4):
                            k = kg * 4 + kj
                            nc.tensor.transpose(
                                p_t[:, kj, :],
                                x_sb[:, k * 128:(k + 1) * 128], ident[:])
                        nc.vector.tensor_copy(
                            xt_buf[:, kg * 4:(kg + 1) * 4,
                                   st4 * 128:(st4 + 1) * 128],
                            p_t[:])

                # Q^T (bias on DVE), K^T (no bias) for this 512-wide chunk
                for wt, dst, has_bias in ((wq_sb, qt, True),
                                          (wk_sb, kt_sb, False)):
                    for m in range(2):
                        p_q = aux.tile([128, 512], F32, tag="aux", name="p_q")
                        for k in range(KT):
                            nc.tensor.matmul(
                                p_q[:], wt[:, k, m * 128:(m + 1) * 128],
                                xt_buf[:, k, :],
                                start=(k == 0), stop=(k == KT - 1))
                        for a in range(2):
                            h = 2 * m + a
                            rows = slice(a * 64, a * 64 + 64)
                            dslice = dst[rows, h, sc * 512:(sc + 1) * 512]
                            if has_bias:
                                nc.vector.tensor_scalar_add(
                                    dslice, p_q[rows, :],
                                    bq_sb[rows, m:m + 1])
                            else:
                                nc.vector.tensor_copy(dslice, p_q[rows, :])

                # V for the 4 s-tiles of this chunk
                for st4 in range(4):
                    sti = sc * 4 + st4
                    p_v = aux.tile([128, DH], F32, tag="aux", name="p_v")
                    for k in range(KT):
                        nc.tensor.matmul(
                            p_v[:], xt_buf[:, k, st4 * 128:(st4 + 1) * 128],
                            wv_sb[:, k, :],
                            start=(k == 0), stop=(k == KT - 1))
                    nc.vector.tensor_add(
                        v_aug[:, sti, :, 0:DK],
                        p_v[:].rearrange("p (h d) -> p h d", h=H_LOC),
                        bv_bc[:].rearrange("p (h d) -> p h d", h=H_LOC))

            # ---------- phase B + interleaved phase C ----------
            for qh in range(QC):
                q0 = qh * 512
                for hm in range(2):
                    po = [ops.tile([DK + 1, 512], F32, tag="po",
                                   name=f"po_{qh}_{hm}_{a}")
                          for a in range(2)]
                    for kti in range(ST):
                        s_ps = sps.tile([128, 1024], F32, tag="ps")
                        for a in range(2):
                            h = 2 * hm + a
                            nc.tensor.matmul(
                                s_ps[:, a * 512:(a + 1) * 512],
                                kt_sb[:, h, kti * 128:(kti + 1) * 128],
                                qt[:, h, q0:q0 + 512],
                                start=True, stop=True)
                        e_t = epool.tile([128, 1024], BF16, tag="et")
                        nc.scalar.activation(
                            e_t[:], s_ps[:], AF.Exp,
                            bias=mask_sb[:, kti:kti + 1], scale=INV_SCALE)
                        for a in range(2):
                            h = 2 * hm + a
                            nc.tensor.matmul(
                                po[a][:],
                                v_aug[:, kti, h, :],
                                e_t[:, a * 512:(a + 1) * 512],
                                start=(kti == 0), stop=(kti == ST - 1),
                                skip_group_check=True)
                    # normalize straight out of PSUM
                    for a in range(2):
                        hp = 64 * a
                        r_sb = small.tile([1, 512], F32, tag="rsb",
                                          name=f"rsb{qh}{hm}{a}")
                        nc.vector.reciprocal(r_sb[0:1, :],
                                             po[a][DK:DK + 1, :])
                        bc_sb = small.tile([64, 512], F32, tag="bcsb",
                                           name=f"bcsb{qh}{hm}{a}")
                        nc.gpsimd.partition_broadcast(
                            bc_sb[:], r_sb[0:1, :], channels=64)
                        nc.vector.tensor_mul(
                            out_ht[hp:hp + 64, hm, q0:q0 + 512],
                            po[a][0:DK, :], bc_sb[:])

                # phase C for this finished q-chunk: the PE chews on these
                # while ACT drains the next block's exps
                for st4 in range(4):
                    sti = qh * 4 + st4
                    y_sb = ypool.tile([128, D], BF16, tag="ysb")
                    for m in range(2):
                        p_y = aux.tile([128, 512], F32, tag="aux", name="p_y")
                        for k2 in range(2):
                            nc.tensor.matmul(
                                p_y[:],
                                out_ht[:, k2, sti * 128:(sti + 1) * 128],
                                w_out_sb[:, k2, m * 512:(m + 1) * 512],
                                start=(k2 == 0), stop=(k2 == 1))
                        nc.vector.tensor_copy(
                            y_sb[:, m * 512:(m + 1) * 512], p_y[:])
                    nc.sync.dma_start(y[sti * 128:(sti + 1) * 128, :],
                                      y_sb[:])

    nc.compile()
    return nc


def kernel(x, mask, W_qkv, b_qkv, W_out, b_out):
    global _CACHED_NC, LAST_EXEC_NS, LAST_RESULTS, LAST_IN_MAPS
    x = np.ascontiguousarray(np.asarray(x, dtype=np.float32))
    mask = np.asarray(mask)
    W_qkv = np.asarray(W_qkv, dtype=np.float32)
    b_qkv = np.asarray(b_qkv, dtype=np.float32)
    W_out = np.ascontiguousarray(np.asarray(W_out, dtype=np.float32))
    b_out_full = np.asarray(b_out, dtype=np.float32)

    B = x.shape[0]
    if _CACHED_NC is None:
        _CACHED_NC = _build()
    nc = _CACHED_NC

    mask_bias = ((mask.astype(np.float32) - 1.0) * 1e9).astype(np.float32)
    ident = np.eye(128, dtype=np.float32)

    in_maps = []
    for c in range(8):
        b = c // 4
        g = c % 4
        cs = g * DH
        in_maps.append({
            "x": x[b],
            "ident": ident,
            "bv_bc": np.broadcast_to(
                b_qkv[2 * D + cs:2 * D + cs + DH], (128, DH)).copy(),
            "w_q": np.ascontiguousarray(W_qkv[:, cs:cs + DH]),
            "w_k": np.ascontiguousarray(W_qkv[:, D + cs:D + cs + DH]),
            "w_v": np.ascontiguousarray(W_qkv[:, 2 * D + cs:2 * D + cs + DH]),
            "b_q": np.ascontiguousarray(b_qkv[cs:cs + DH]),
            "w_out": np.ascontiguousarray(W_out[cs:cs + DH, :]),
            "mask_bias": mask_bias[b],
        })

    kwargs = {}
    if TRACE:
        kwargs["trace"] = True
        if TRACE_ALL_CORES:
            kwargs["trace_cores"] = list(range(8))
    LAST_IN_MAPS = in_maps
    res = None
    for attempt in range(3):
        try:
            res = run_bass_kernel_spmd(nc, in_maps, core_ids=list(range(8)),
                                       **kwargs)
            break
        except Exception:
            if attempt == 2:
                raise
            import time as _time
            _time.sleep(2.0)
    LAST_EXEC_NS = res.exec_time_ns
    LAST_RESULTS = res

    out = np.zeros((B, S, D), dtype=np.float32)
    for c in range(8):
        out[c // 4] += np.asarray(res.results[c]["y"]).astype(np.float32)
    out += b_out_full
    return out
